# revision 2
# baseline (speedup 1.0000x reference)
"""Behavior-specific FFN (MoE routing) Trainium2 kernel.

Strategy: expert-parallel with host-side routing. Tokens are gathered by
behavior id on the host (numpy), each behavior's tokens are split across
2 of the 8 NeuronCores, and every core runs a dense 2-layer FFN
(relu(x @ W1 + B1) @ W2 + B2) for its single behavior over its token
shard. The host scatters results back; padding tokens (behavior 0) stay
zero.

Device layout: tokens live on the matmul free (moving) dim, feature dims
on partitions. Layer 1: out[F_tile, tok] += W1[H_tile, F_tile].T @
xT[H_tile, tok]; layer 2 contracts over F the same way. x is fed
pre-transposed ([H, N]) by the host so no on-device transpose is needed.
"""

import numpy as np

_B, _T, _H, _F = 32, 512, 512, 2048
_NB = 4
_P = 128
_NCORES = 8
_TOK_TILE = 512

# Stash of the most recent BassKernelResults (exec_time_ns etc.) for the
# local test harness; harmless in the grading path.
LAST_RESULTS = None

_NC_CACHE = {}


def _token_tiles(n_pad):
    """Chunk n_pad into token tiles, every tile in [256, 512] columns.

    fp32r matmuls only run at full rate with a moving dim >= 256, so the
    tail is split into two roughly-equal tiles instead of leaving a
    narrow remainder. n_pad itself is exact (no alignment padding)."""
    assert n_pad >= 256
    tiles = []
    off = 0
    rem = n_pad
    while rem > 1024:
        tiles.append((off, _TOK_TILE))
        off += _TOK_TILE
        rem -= _TOK_TILE
    if rem > 512:
        a = ((rem // 2 + 15) // 16) * 16
        tiles.append((off, a))
        tiles.append((off + a, rem - a))
    else:
        tiles.append((off, rem))
    return tiles


def _dedupe_ldweights(nc):
    """Remove Ldweights that re-load the exact weights already resident in
    the PE array (same AP, no intervening clobber, no sync conditions).
    The paired Matmults (ldweights=False) then use the already-loaded
    weights — this is the documented explicit-LDW + non-self-loading-MM
    hardware pattern (valid for bf16; NOT for fp32/fp32r)."""
    removed = 0
    for f in nc.m.functions:
        for blk in f.blocks:
            keep = []
            last_key = None
            for inst in blk.instructions:
                op = inst.opcode
                if op == "Ldweights":
                    x = inst.ins[0]
                    key = (
                        getattr(x, "memref", None),
                        str(getattr(x, "ap", None)),
                        getattr(x, "offset", None),
                    )
                    clean = not (inst.has_wait() or inst.has_update())
                    if clean and key == last_key:
                        removed += 1
                        continue
                    last_key = key
                elif op in ("Matmult", "EventSemaphore", "Nop", "Activation",
                            "TensorCopy", "TensorTensor", "TensorScalarPtr",
                            "DMACopy", "TensorReduce", "Memset"):
                    pass  # doesn't clobber the PE weight array
                else:
                    last_key = None
                keep.append(inst)
            if removed:
                blk.instructions[:] = keep
    return removed


def _build_v2(n_pad, mm_dtype_name, repeats=1, loop=1):
    """fp16-oriented stream kernel with DMA ordering tuned for both the cold
    single pass and the steady-state loop.

    - x/W/h/y all in mm_dt (fp16): halves DMA traffic vs fp32; PE rate is
      identical (1 row/cycle) and rel err ~4e-4 vs the 2e-2 gate.
    - Cold pass: the first token tile's x is DMA'd before the weights, and
      weight chunks arrive in consumption order, so the PE starts ~2.4 us in
      instead of ~29 us (all DMAs share one ~330 GB/s pipe, so order is
      everything).
    - Weight chunks are >=1 KiB per descriptor (512-col slices); 128-col fp16
      slices would run the DMA at half bandwidth.
    - y out-DMAs are issued per m2-group from the Activation HWDGE queue:
      smaller iteration tail, and a second HW queue on real silicon.
    """
    from contextlib import ExitStack, nullcontext

    import concourse.bass as bass
    import concourse.mybir as mybir
    import concourse.tile as tile
    from concourse import bacc

    f32 = mybir.dt.float32
    mm_dt = getattr(mybir.dt, mm_dtype_name)
    AF = mybir.ActivationFunctionType
    KH = _H // _P   # 4  K-subtiles for layer 1 / M-tiles for layer 2
    MF = _F // _P   # 16 M-tiles for layer 1 / K-subtiles for layer 2

    nc = bacc.Bacc("TRN2", target_bir_lowering=False, debug=False, num_devices=_NCORES)
    xT = nc.dram_tensor("xT", [_H, n_pad], mm_dt, kind="ExternalInput").ap()
    w1 = nc.dram_tensor("w1", [_H, _F], mm_dt, kind="ExternalInput").ap()
    w2 = nc.dram_tensor("w2", [_F, _H], mm_dt, kind="ExternalInput").ap()
    b1 = nc.dram_tensor("b1", [_P, MF], f32, kind="ExternalInput").ap()
    b2 = nc.dram_tensor("b2", [_P, KH], f32, kind="ExternalInput").ap()
    yT = nc.dram_tensor("yT", [_H, n_pad], mm_dt, kind="ExternalOutput").ap()

    with tile.TileContext(nc) as tc, ExitStack() as ctx:
        consts = ctx.enter_context(tc.tile_pool(name="consts", bufs=1))
        xp = ctx.enter_context(tc.tile_pool(name="xp", bufs=3))
        hp = ctx.enter_context(tc.tile_pool(name="hp", bufs=2))
        yp = ctx.enter_context(tc.tile_pool(name="yp", bufs=3))
        pp = ctx.enter_context(tc.tile_pool(name="pp", bufs=4, space="PSUM"))

        w1s = consts.tile([_P, KH, _F], mm_dt)
        w2s = consts.tile([_P, MF, _H], mm_dt)
        b1s = consts.tile([_P, MF], f32)
        b2s = consts.tile([_P, KH], f32)
        w1r = w1.rearrange("(ko p) f -> p ko f", p=_P)
        w2r = w2.rearrange("(ko p) h -> p ko h", p=_P)
        xTr = xT.rearrange("(ko p) n -> p ko n", p=_P)
        yTr = yT.rearrange("(mo p) n -> p mo n", p=_P)

        def load_weights(first_chunk_only=False, rest_only=False):
            # 512-col w1 chunks: chunk c covers m-groups 4c..4c+3 in
            # consumption order. w2 is needed only from layer 2 (~45 us in).
            if not rest_only:
                nc.sync.dma_start(w1s[:, :, 0:512], w1r[:, :, 0:512])
                if first_chunk_only:
                    return
            nc.sync.dma_start(b1s[:], b1)
            nc.sync.dma_start(b2s[:], b2)
            for c in range(1, 4):
                nc.sync.dma_start(
                    w1s[:, :, c * 512:(c + 1) * 512], w1r[:, :, c * 512:(c + 1) * 512]
                )
            for c in range(2):
                nc.sync.dma_start(w2s[:, c * 8:(c + 1) * 8, :], w2r[:, c * 8:(c + 1) * 8, :])

        cold = loop == 1 and repeats == 1
        if not cold:
            load_weights()

        tiles = _token_tiles(n_pad)
        loop_cm = (
            tc.For_i(0, loop, 1, hint_engines=(mybir.EngineType.PE, mybir.EngineType.Activation, mybir.EngineType.SP))
            if loop > 1
            else nullcontext()
        )
        with loop_cm:
          for _rep in range(repeats):
            for ti, (t0, tn) in enumerate(tiles):
                sl = slice(t0, t0 + tn)
                xt = xp.tile([_P, KH, tn], mm_dt, tag="xt")
                nc.sync.dma_start(xt[:, 0, :], xTr[:, 0, sl])
                if cold and ti == 0:
                    # First w1 chunk races the remaining x chunks to the PE.
                    load_weights(first_chunk_only=True)
                for k in range(1, KH):
                    nc.sync.dma_start(xt[:, k, :], xTr[:, k, sl])
                if cold and ti == 0:
                    load_weights(rest_only=True)

                ht = hp.tile([_P, MF, tn], mm_dt, tag="ht")
                for m in range(MF):
                    ps = pp.tile([_P, tn], f32, tag="ps1", name="ps1")
                    for k in range(KH):
                        nc.tensor.matmul(
                            ps[:],
                            w1s[:, k, m * _P:(m + 1) * _P],
                            xt[:, k, :],
                            start=(k == 0),
                            stop=(k == KH - 1),
                        )
                    nc.scalar.activation(ht[:, m, :], ps[:], AF.Relu, bias=b1s[:, m:m + 1])

                yt = yp.tile([_P, KH, tn], mm_dt, tag="yt")
                for m2 in range(KH):
                    ps2 = pp.tile([_P, tn], f32, tag="ps2", name="ps2")
                    for k2 in range(MF):
                        nc.tensor.matmul(
                            ps2[:],
                            w2s[:, k2, m2 * _P:(m2 + 1) * _P],
                            ht[:, k2, :],
                            start=(k2 == 0),
                            stop=(k2 == MF - 1),
                        )
                    nc.scalar.activation(yt[:, m2, :], ps2[:], AF.Identity, bias=b2s[:, m2:m2 + 1])
                    nc.scalar.dma_start(yTr[:, m2, sl], yt[:, m2, :])

    nc.compile()
    return nc


def _build(n_pad, mm_dtype_name, repeats=1, loop=1, style="stream"):
    if style == "v2":
        return _build_v2(n_pad, mm_dtype_name, repeats, loop)
    import os as _os
    _dev = _os.environ.get("BSPFF_DEV") == "1"
    sub_n = int(_os.environ.get("SUB_N", "512")) if _dev else 512
    skip_act = _dev and _os.environ.get("SKIP_ACT") == "1"
    skip_ydma = _dev and _os.environ.get("SKIP_YDMA") == "1"
    skip_xdma = _dev and _os.environ.get("SKIP_XDMA") == "1"
    evac = _os.environ.get("EVAC", "act") if _dev else "act"
    w_in_loop = _dev and _os.environ.get("W_IN_LOOP") == "1"

    from contextlib import ExitStack, nullcontext

    import concourse.bass as bass
    import concourse.mybir as mybir
    import concourse.tile as tile
    from concourse import bacc

    f32 = mybir.dt.float32
    mm_dt = getattr(mybir.dt, mm_dtype_name)
    AF = mybir.ActivationFunctionType
    KH = _H // _P   # 4  K-subtiles for layer 1 / M-tiles for layer 2
    MF = _F // _P   # 16 M-tiles for layer 1 / K-subtiles for layer 2

    nc = bacc.Bacc("TRN2", target_bir_lowering=False, debug=False, num_devices=_NCORES)
    xT = nc.dram_tensor("xT", [_H, n_pad], mm_dt, kind="ExternalInput").ap()
    w1 = nc.dram_tensor("w1", [_H, _F], mm_dt, kind="ExternalInput").ap()
    w2 = nc.dram_tensor("w2", [_F, _H], mm_dt, kind="ExternalInput").ap()
    b1 = nc.dram_tensor("b1", [_P, MF], f32, kind="ExternalInput").ap()
    b2 = nc.dram_tensor("b2", [_P, KH], f32, kind="ExternalInput").ap()
    yT = nc.dram_tensor("yT", [_H, n_pad], f32, kind="ExternalOutput").ap()

    with tile.TileContext(nc) as tc, ExitStack() as ctx:
        grouped = style == "grouped"
        consts = ctx.enter_context(tc.tile_pool(name="consts", bufs=1))
        xp = ctx.enter_context(tc.tile_pool(name="xp", bufs=2 if grouped else 3))
        hp = ctx.enter_context(tc.tile_pool(name="hp", bufs=1 if grouped else 2))
        yp = ctx.enter_context(tc.tile_pool(name="yp", bufs=2 if grouped else 3))
        import os as _os2
        _onetag = _os2.environ.get("PSUM_ONETAG") == "1" and _os.environ.get("BSPFF_DEV") == "1"
        pp = ctx.enter_context(tc.tile_pool(name="pp", bufs=8 if (grouped or _onetag) else 4, space="PSUM"))

        w1s = consts.tile([_P, KH, _F], mm_dt)
        w2s = consts.tile([_P, MF, _H], mm_dt)
        w1r = w1.rearrange("(ko p) f -> p ko f", p=_P)
        w2r = w2.rearrange("(ko p) h -> p ko h", p=_P)

        def load_weights():
            # Chunk weight loads by output-column slice: the m-th matmul group
            # only needs its own 128-wide slice, so compute starts ~1-2 us in.
            # W2 chunks are interleaved between W1 chunks so layer-2's first
            # weights aren't queued behind all 4 MB of W1 on the DMA queues
            # (layer 2 of token-tile 0 starts ~17 us into a cold pass).
            for m in range(MF):
                nc.sync.dma_start(w1s[:, :, m * _P:(m + 1) * _P], w1r[:, :, m * _P:(m + 1) * _P])
                if m % 4 == 3:
                    m2 = m // 4
                    nc.sync.dma_start(w2s[:, :, m2 * _P:(m2 + 1) * _P], w2r[:, :, m2 * _P:(m2 + 1) * _P])

        if not w_in_loop:
            load_weights()
        b1s = consts.tile([_P, MF], f32)
        nc.sync.dma_start(b1s[:], b1)
        b2s = consts.tile([_P, KH], f32)
        nc.sync.dma_start(b2s[:], b2)

        xTr = xT.rearrange("(ko p) n -> p ko n", p=_P)
        yTr = yT.rearrange("(mo p) n -> p mo n", p=_P)

        tiles = _token_tiles(n_pad)
        loop_cm = (
            tc.For_i(0, loop, 1, hint_engines=(mybir.EngineType.PE, mybir.EngineType.Activation, mybir.EngineType.SP))
            if loop > 1
            else nullcontext()
        )
        with loop_cm:
          for _rep in range(repeats):
            if w_in_loop:
                load_weights()
            if style == "grouped":
                # Tokens-inner order: one weight tile feeds all token tiles
                # back-to-back, so redundant Ldweights can be dropped.
                xt = xp.tile([_P, KH, n_pad], mm_dt, tag="xt")
                for k in range(KH):
                    nc.sync.dma_start(xt[:, k, :], xTr[:, k, :])
                ht = hp.tile([_P, MF, n_pad], mm_dt, tag="ht")
                for m in range(MF):
                    pss = [
                        pp.tile([_P, tn], f32, tag="ps", name=f"ps_{m}_{i}")
                        for i, (t0, tn) in enumerate(tiles)
                    ]
                    for k in range(KH):
                        for i, (t0, tn) in enumerate(tiles):
                            nc.tensor.matmul(
                                pss[i][:],
                                w1s[:, k, m * _P:(m + 1) * _P],
                                xt[:, k, t0:t0 + tn],
                                start=(k == 0),
                                stop=(k == KH - 1),
                            )
                    for i, (t0, tn) in enumerate(tiles):
                        nc.scalar.activation(
                            ht[:, m, t0:t0 + tn], pss[i][:], AF.Relu, bias=b1s[:, m:m + 1]
                        )
                yt = yp.tile([_P, KH, n_pad], f32, tag="yt")
                for m2 in range(KH):
                    pss2 = [
                        pp.tile([_P, tn], f32, tag="ps", name=f"ps2_{m2}_{i}")
                        for i, (t0, tn) in enumerate(tiles)
                    ]
                    for k2 in range(MF):
                        for i, (t0, tn) in enumerate(tiles):
                            nc.tensor.matmul(
                                pss2[i][:],
                                w2s[:, k2, m2 * _P:(m2 + 1) * _P],
                                ht[:, k2, t0:t0 + tn],
                                start=(k2 == 0),
                                stop=(k2 == MF - 1),
                            )
                    for i, (t0, tn) in enumerate(tiles):
                        nc.scalar.activation(
                            yt[:, m2, t0:t0 + tn], pss2[i][:], AF.Identity, bias=b2s[:, m2:m2 + 1]
                        )
                for m2 in range(KH):
                    nc.sync.dma_start(yTr[:, m2, :], yt[:, m2, :])
                continue

            for t0, tn in tiles:
                sl = slice(t0, t0 + tn)
                nq = (tn + sub_n - 1) // sub_n
                xt = xp.tile([_P, KH, tn], mm_dt, tag="xt")
                if not skip_xdma:
                    # Chunked by k-subtile: 4 queues in parallel, and matmul
                    # group (m=0, k=0) only waits on the first quarter.
                    for k in range(KH):
                        nc.sync.dma_start(xt[:, k, :], xTr[:, k, sl])

                ht = hp.tile([_P, MF, tn], mm_dt, tag="ht")
                for m in range(MF):
                    ps = pp.tile([_P, tn], f32, tag="ps" if _onetag else "ps1", name="ps1")
                    for k in range(KH):
                        for q in range(nq):
                            qs = slice(q * sub_n, min((q + 1) * sub_n, tn))
                            nc.tensor.matmul(
                                ps[:, qs],
                                w1s[:, k, m * _P:(m + 1) * _P],
                                xt[:, k, qs],
                                start=(k == 0 and q == 0),
                                stop=(k == KH - 1 and q == nq - 1),
                                skip_group_check=True,
                            )
                    if not skip_act:
                        use_dve = evac == "dve" or (evac == "split" and m % 2 == 1)
                        if use_dve:
                            nc.vector.tensor_scalar(
                                ht[:, m, :], ps[:], b1s[:, m:m + 1], 0.0,
                                mybir.AluOpType.add, mybir.AluOpType.max,
                            )
                        else:
                            nc.scalar.activation(ht[:, m, :], ps[:], AF.Relu, bias=b1s[:, m:m + 1])

                yt = yp.tile([_P, KH, tn], f32, tag="yt")
                for m2 in range(KH):
                    ps2 = pp.tile([_P, tn], f32, tag="ps" if _onetag else "ps2", name="ps2")
                    for k2 in range(MF):
                        for q in range(nq):
                            qs = slice(q * sub_n, min((q + 1) * sub_n, tn))
                            nc.tensor.matmul(
                                ps2[:, qs],
                                w2s[:, k2, m2 * _P:(m2 + 1) * _P],
                                ht[:, k2, qs],
                                start=(k2 == 0 and q == 0),
                                stop=(k2 == MF - 1 and q == nq - 1),
                                skip_group_check=True,
                            )
                    if not skip_act:
                        use_dve = evac == "dve" or (evac == "split" and m2 % 2 == 1)
                        if use_dve:
                            nc.vector.tensor_scalar_add(yt[:, m2, :], ps2[:], b2s[:, m2:m2 + 1])
                        else:
                            nc.scalar.activation(yt[:, m2, :], ps2[:], AF.Identity, bias=b2s[:, m2:m2 + 1])
                if not skip_ydma:
                    nc.sync.dma_start(yTr[:, :, sl], yt[:])

    nc.compile()
    if style == "grouped":
        n_removed = _dedupe_ldweights(nc)
        import os as _os
        if _os.environ.get("DEBUG_LDW"):
            print(f"[kernel] deduped {n_removed} Ldweights")
    return nc


_MM_DTYPE = "float32r"
_MM_STYLE = "stream"


def _get_nc(n_pad, mm_dtype_name, repeats=1, loop=1, style="stream"):
    key = (n_pad, mm_dtype_name, repeats, loop, style)
    if key not in _NC_CACHE:
        _NC_CACHE[key] = _build(n_pad, mm_dtype_name, repeats, loop, style)
    return _NC_CACHE[key]


def _np_mm_dtype(mm_dtype_name):
    if mm_dtype_name == "bfloat16":
        import ml_dtypes

        return ml_dtypes.bfloat16
    if mm_dtype_name == "float16":
        return np.float16
    return np.float32


def _prepare(x, b_seq, W1, B1, W2, B2, mm_dtype_name):
    """Host-side routing: returns (idx_per_core, n_pad, in_maps)."""
    np_dt = _np_mm_dtype(mm_dtype_name)
    x = np.asarray(x)
    flat_x = np.ascontiguousarray(x.reshape(-1, _H), dtype=np.float32)
    bs = np.asarray(b_seq).reshape(-1)

    # Route: behavior b -> cores 2b and 2b+1, tokens split evenly.
    idx_per_core = []
    for b in range(_NB):
        idx = np.nonzero(bs == b + 1)[0]
        h = (len(idx) + 1) // 2
        idx_per_core.append(idx[:h])
        idx_per_core.append(idx[h:])
    nmax = max(len(i) for i in idx_per_core)
    n_pad = ((max(256, nmax) + 15) // 16) * 16

    in_maps = []
    for c in range(_NCORES):
        beh = c // 2
        idx = idx_per_core[c]
        xT = np.zeros((_H, n_pad), np_dt)
        if len(idx):
            xT[:, :len(idx)] = flat_x[idx].T.astype(np_dt)
        in_maps.append({
            "xT": xT,
            "w1": np.ascontiguousarray(np.asarray(W1[beh]).astype(np_dt)),
            "w2": np.ascontiguousarray(np.asarray(W2[beh]).astype(np_dt)),
            "b1": np.ascontiguousarray(np.asarray(B1[beh], dtype=np.float32).reshape(_F // _P, _P).T),
            "b2": np.ascontiguousarray(np.asarray(B2[beh], dtype=np.float32).reshape(_H // _P, _P).T),
        })
    return idx_per_core, n_pad, in_maps


def kernel(x, b_seq, W1, B1, W2, B2, _repeats=1):
    global LAST_RESULTS
    import os

    from concourse.bass_utils import run_bass_kernel_spmd

    _dev = os.environ.get("BSPFF_DEV") == "1"
    mm_dtype = os.environ.get("MM_DTYPE", _MM_DTYPE) if _dev else _MM_DTYPE
    style = os.environ.get("MM_STYLE", _MM_STYLE) if _dev else _MM_STYLE
    idx_per_core, n_pad, in_maps = _prepare(x, b_seq, W1, B1, W2, B2, mm_dtype)
    nc = _get_nc(n_pad, mm_dtype, _repeats, style=style)

    res = run_bass_kernel_spmd(nc, in_maps, core_ids=list(range(_NCORES)))
    LAST_RESULTS = res

    out = np.zeros((_B * _T, _H), np.float32)
    for c in range(_NCORES):
        idx = idx_per_core[c]
        if len(idx):
            out[idx] = res.results[c]["yT"][:, :len(idx)].T
    return out.reshape(_B, _T, _H)



# revision 11
# speedup vs baseline: 3.2571x; 3.2571x over previous
"""Behavior-specific FFN (MoE routing) Trainium2 kernel.

Strategy: expert-parallel with host-side routing. Tokens are gathered by
behavior id on the host (numpy), each behavior's tokens are split across
2 of the 8 NeuronCores, and every core runs a dense 2-layer FFN
(relu(x @ W1 + B1) @ W2 + B2) for its single behavior over its token
shard. The host scatters results back; padding tokens (behavior 0) stay
zero.

Device layout: tokens live on the matmul free (moving) dim, feature dims
on partitions. Layer 1: out[F_tile, tok] += W1[H_tile, F_tile].T @
xT[H_tile, tok]; layer 2 contracts over F the same way. x is fed
pre-transposed ([H, N]) by the host so no on-device transpose is needed.
"""

import numpy as np

_B, _T, _H, _F = 32, 512, 512, 2048
_NB = 4
_P = 128
_NCORES = 8
_TOK_TILE = 512

# Stash of the most recent BassKernelResults (exec_time_ns etc.) for the
# local test harness; harmless in the grading path.
LAST_RESULTS = None

_NC_CACHE = {}


def _token_tiles(n_pad):
    """Chunk n_pad into token tiles, every tile in [256, 512] columns.

    fp32r matmuls only run at full rate with a moving dim >= 256, so the
    tail is split into two roughly-equal tiles instead of leaving a
    narrow remainder. n_pad itself is exact (no alignment padding)."""
    assert n_pad >= 256
    tiles = []
    off = 0
    rem = n_pad
    while rem > 1024:
        tiles.append((off, _TOK_TILE))
        off += _TOK_TILE
        rem -= _TOK_TILE
    if rem > 512:
        a = ((rem // 2 + 15) // 16) * 16
        tiles.append((off, a))
        tiles.append((off + a, rem - a))
    else:
        tiles.append((off, rem))
    return tiles


def _dedupe_ldweights(nc, move_waits=False):
    """Remove Ldweights that re-load the exact weights already resident in
    the PE array (same AP, no intervening clobber, no sync conditions).
    The paired Matmults (ldweights=False) then use the already-loaded
    weights — this is the documented explicit-LDW + non-self-loading-MM
    hardware pattern (valid for bf16/fp16; NOT for fp32/fp32r).

    (move_waits is reserved; waits currently pin an Ldweights in place.)"""
    removed = 0
    for f in nc.m.functions:
        for blk in f.blocks:
            keep = []
            last_key = None
            insts = blk.instructions
            for inst in insts:
                op = inst.opcode
                if op == "Ldweights":
                    x = inst.ins[0]
                    key = (
                        getattr(x, "memref", None),
                        str(getattr(x, "ap", None)),
                        getattr(x, "offset", None),
                    )
                    clean = not (inst.has_wait() or inst.has_update())
                    if clean and key == last_key:
                        removed += 1
                        continue
                    last_key = key
                elif op in ("Matmult", "EventSemaphore", "Nop", "Activation",
                            "TensorCopy", "TensorTensor", "TensorScalarPtr",
                            "DMACopy", "TensorReduce", "Memset"):
                    pass  # doesn't clobber the PE weight array
                else:
                    last_key = None
                keep.append(inst)
            if removed:
                blk.instructions[:] = keep
    return removed


def _build_v2(n_pad, mm_dtype_name, repeats=1, loop=1):
    """fp16-oriented stream kernel with DMA ordering tuned for both the cold
    single pass and the steady-state loop.

    - x/W/h/y all in mm_dt (fp16): halves DMA traffic vs fp32; PE rate is
      identical (1 row/cycle) and rel err ~4e-4 vs the 2e-2 gate.
    - Cold pass: the first token tile's x is DMA'd before the weights, and
      weight chunks arrive in consumption order, so the PE starts ~2.4 us in
      instead of ~29 us (all DMAs share one ~330 GB/s pipe, so order is
      everything).
    - Weight chunks are >=1 KiB per descriptor (512-col slices); 128-col fp16
      slices would run the DMA at half bandwidth.
    - y out-DMAs are issued per m2-group from the Activation HWDGE queue:
      smaller iteration tail, and a second HW queue on real silicon.
    """
    from contextlib import ExitStack, nullcontext

    import concourse.bass as bass
    import concourse.mybir as mybir
    import concourse.tile as tile
    from concourse import bacc

    f32 = mybir.dt.float32
    mm_dt = getattr(mybir.dt, mm_dtype_name)
    AF = mybir.ActivationFunctionType
    KH = _H // _P   # 4  K-subtiles for layer 1 / M-tiles for layer 2
    MF = _F // _P   # 16 M-tiles for layer 1 / K-subtiles for layer 2

    nc = bacc.Bacc("TRN2", target_bir_lowering=False, debug=False, num_devices=_NCORES)
    xT = nc.dram_tensor("xT", [_H, n_pad], mm_dt, kind="ExternalInput").ap()
    w1 = nc.dram_tensor("w1", [_H, _F], mm_dt, kind="ExternalInput").ap()
    w2 = nc.dram_tensor("w2", [_F, _H], mm_dt, kind="ExternalInput").ap()
    b1 = nc.dram_tensor("b1", [_P, MF], f32, kind="ExternalInput").ap()
    b2 = nc.dram_tensor("b2", [_P, KH], f32, kind="ExternalInput").ap()
    yT = nc.dram_tensor("yT", [_H, n_pad], mm_dt, kind="ExternalOutput").ap()

    with tile.TileContext(nc) as tc, ExitStack() as ctx:
        consts = ctx.enter_context(tc.tile_pool(name="consts", bufs=1))
        xp = ctx.enter_context(tc.tile_pool(name="xp", bufs=3))
        hp = ctx.enter_context(tc.tile_pool(name="hp", bufs=2))
        yp = ctx.enter_context(tc.tile_pool(name="yp", bufs=3))
        pp = ctx.enter_context(tc.tile_pool(name="pp", bufs=4, space="PSUM"))

        w1s = consts.tile([_P, KH, _F], mm_dt)
        w2s = consts.tile([_P, MF, _H], mm_dt)
        b1s = consts.tile([_P, MF], f32)
        b2s = consts.tile([_P, KH], f32)
        w1r = w1.rearrange("(ko p) f -> p ko f", p=_P)
        w2r = w2.rearrange("(ko p) h -> p ko h", p=_P)
        xTr = xT.rearrange("(ko p) n -> p ko n", p=_P)
        yTr = yT.rearrange("(mo p) n -> p mo n", p=_P)

        def load_weights(first_chunk_only=False, rest_only=False):
            # 512-col w1 chunks: chunk c covers m-groups 4c..4c+3 in
            # consumption order. w2 is needed only from layer 2 (~45 us in).
            if not rest_only:
                nc.sync.dma_start(w1s[:, :, 0:512], w1r[:, :, 0:512])
                if first_chunk_only:
                    return
            nc.sync.dma_start(b1s[:], b1)
            nc.sync.dma_start(b2s[:], b2)
            for c in range(1, 4):
                nc.sync.dma_start(
                    w1s[:, :, c * 512:(c + 1) * 512], w1r[:, :, c * 512:(c + 1) * 512]
                )
            for c in range(2):
                nc.sync.dma_start(w2s[:, c * 8:(c + 1) * 8, :], w2r[:, c * 8:(c + 1) * 8, :])

        cold = loop == 1 and repeats == 1
        if not cold:
            load_weights()

        tiles = _token_tiles(n_pad)
        loop_cm = (
            tc.For_i(0, loop, 1, hint_engines=(mybir.EngineType.PE, mybir.EngineType.Activation, mybir.EngineType.SP))
            if loop > 1
            else nullcontext()
        )
        with loop_cm:
          for _rep in range(repeats):
            for ti, (t0, tn) in enumerate(tiles):
                sl = slice(t0, t0 + tn)
                xt = xp.tile([_P, KH, tn], mm_dt, tag="xt")
                nc.sync.dma_start(xt[:, 0, :], xTr[:, 0, sl])
                if cold and ti == 0:
                    # First w1 chunk races the remaining x chunks to the PE.
                    load_weights(first_chunk_only=True)
                for k in range(1, KH):
                    nc.sync.dma_start(xt[:, k, :], xTr[:, k, sl])
                if cold and ti == 0:
                    load_weights(rest_only=True)

                ht = hp.tile([_P, MF, tn], mm_dt, tag="ht")
                for m in range(MF):
                    ps = pp.tile([_P, tn], f32, tag="ps1", name="ps1")
                    for k in range(KH):
                        nc.tensor.matmul(
                            ps[:],
                            w1s[:, k, m * _P:(m + 1) * _P],
                            xt[:, k, :],
                            start=(k == 0),
                            stop=(k == KH - 1),
                        )
                    nc.scalar.activation(ht[:, m, :], ps[:], AF.Relu, bias=b1s[:, m:m + 1])

                yt = yp.tile([_P, KH, tn], mm_dt, tag="yt")
                for m2 in range(KH):
                    ps2 = pp.tile([_P, tn], f32, tag="ps2", name="ps2")
                    for k2 in range(MF):
                        nc.tensor.matmul(
                            ps2[:],
                            w2s[:, k2, m2 * _P:(m2 + 1) * _P],
                            ht[:, k2, :],
                            start=(k2 == 0),
                            stop=(k2 == MF - 1),
                        )
                    nc.scalar.activation(yt[:, m2, :], ps2[:], AF.Identity, bias=b2s[:, m2:m2 + 1])
                    nc.scalar.dma_start(yTr[:, m2, sl], yt[:, m2, :])

    nc.compile()
    return nc


def _v3_token_tiles(n_pad):
    """Token tiles for v3: <=512 (one PSUM bank), 16-aligned, with a small
    final tile so the last evac+DMA tail is short. All widths >=256 keep
    fp32r at full rate should the dtype ever change."""
    assert n_pad % 16 == 0 and n_pad >= 1024
    # [512, ..., 512, mid, 256] with mid in [256, 512]: full tiles lead (the
    # cold pass hides later x DMA behind tile-0 compute), and the small tail
    # tile shortens the final evac+DMA.
    k = (n_pad - 512) // 512
    mid = n_pad - 256 - 512 * k
    assert 256 <= mid <= 512, (n_pad, mid)
    widths = [512] * k + [mid, 256]
    tiles = []
    off = 0
    for w in widths:
        tiles.append((off, w))
        off += w
    return tiles


def _build_v3(n_pad, mm_dtype_name, repeats=1, loop=1):
    """Weight-stationary (tokens-inner) fp16 kernel with cold-start DMA
    orchestration.

    MM order visits all token tiles back-to-back for each weight tile, so
    post-compile dedupe drops redundant Ldweights (~132 remain, the
    structural minimum) — explicit Ldweights cost ~25ns each of PE issue
    time on HW. x is DMA'd per (k, tile) chunk interleaved with weight
    chunks in consumption order so the cold pass starts the PE ~1.2us in.
    y leaves per (m2, tile) from the Activation HWDGE queue right after its
    evacuation, keeping the tail short."""
    from contextlib import ExitStack, nullcontext

    import concourse.bass as bass
    import concourse.mybir as mybir
    import concourse.tile as tile
    from concourse import bacc

    f32 = mybir.dt.float32
    mm_dt = getattr(mybir.dt, mm_dtype_name)
    AF = mybir.ActivationFunctionType
    KH = _H // _P   # 4
    MF = _F // _P   # 16

    nc = bacc.Bacc("TRN2", target_bir_lowering=False, debug=False, num_devices=_NCORES)
    xT = nc.dram_tensor("xT", [_H, n_pad], mm_dt, kind="ExternalInput").ap()
    w1 = nc.dram_tensor("w1", [_H, _F], mm_dt, kind="ExternalInput").ap()
    w2 = nc.dram_tensor("w2", [_F, _H], mm_dt, kind="ExternalInput").ap()
    b1 = nc.dram_tensor("b1", [_P, MF], f32, kind="ExternalInput").ap()
    b2 = nc.dram_tensor("b2", [_P, KH], f32, kind="ExternalInput").ap()
    yT = nc.dram_tensor("yT", [_H, n_pad], mm_dt, kind="ExternalOutput").ap()

    tiles = _v3_token_tiles(n_pad)
    NT = len(tiles)

    with tile.TileContext(nc) as tc, ExitStack() as ctx:
        consts = ctx.enter_context(tc.tile_pool(name="consts", bufs=1))
        xp = ctx.enter_context(tc.tile_pool(name="xp", bufs=2))
        hp = ctx.enter_context(tc.tile_pool(name="hp", bufs=2))
        yp = ctx.enter_context(tc.tile_pool(name="yp", bufs=3))
        pp = ctx.enter_context(tc.tile_pool(name="pp", bufs=2 * NT, space="PSUM"))

        w1s = consts.tile([_P, KH, _F], mm_dt)
        w2s = consts.tile([_P, MF, _H], mm_dt)
        b1s = consts.tile([_P, MF], f32)
        b2s = consts.tile([_P, KH], f32)
        w1r = w1.rearrange("(ko p) f -> p ko f", p=_P)
        w2r = w2.rearrange("(ko p) h -> p ko h", p=_P)
        xTr = xT.rearrange("(ko p) n -> p ko n", p=_P)
        yTr = yT.rearrange("(mo p) n -> p mo n", p=_P)

        def w1_chunk(c0, c1):
            nc.sync.dma_start(w1s[:, :, c0:c1], w1r[:, :, c0:c1])

        def load_weights_tail():
            nc.sync.dma_start(b1s[:], b1)
            nc.sync.dma_start(b2s[:], b2)
            for c in range(1, 4):
                w1_chunk(c * 512, (c + 1) * 512)
            for c in range(2):
                nc.sync.dma_start(w2s[:, c * 8:(c + 1) * 8, :], w2r[:, c * 8:(c + 1) * 8, :])

        cold = loop == 1 and repeats == 1
        if not cold:
            w1_chunk(0, 512)
            load_weights_tail()

        loop_cm = (
            tc.For_i(0, loop, 1, hint_engines=(mybir.EngineType.PE, mybir.EngineType.Activation, mybir.EngineType.SP))
            if loop > 1
            else nullcontext()
        )
        with loop_cm:
          for _rep in range(repeats):
            xt = xp.tile([_P, KH, n_pad], mm_dt, tag="xt")
            # x arrives per (k, tile) chunk. Cold pass: tile-0's k-chunks and
            # the first weight chunks lead, in the hybrid consumption order.
            if cold:
                t0_, tn_ = tiles[0]
                nc.sync.dma_start(xt[:, 0, t0_:t0_ + tn_], xTr[:, 0, t0_:t0_ + tn_])
                w1_chunk(0, 128)
                for k in range(1, KH):
                    nc.sync.dma_start(xt[:, k, t0_:t0_ + tn_], xTr[:, k, t0_:t0_ + tn_])
                nc.sync.dma_start(b1s[:], b1)
                w1_chunk(128, 512)
                for i, (t0, tn) in enumerate(tiles[1:], 1):
                    for k in range(KH):
                        nc.sync.dma_start(xt[:, k, t0:t0 + tn], xTr[:, k, t0:t0 + tn])
                    w1_chunk(i * 512, (i + 1) * 512)
                nc.sync.dma_start(b2s[:], b2)
                for c in range(2):
                    nc.sync.dma_start(w2s[:, c * 8:(c + 1) * 8, :], w2r[:, c * 8:(c + 1) * 8, :])
            else:
                for k in range(KH):
                    for i, (t0, tn) in enumerate(tiles):
                        nc.sync.dma_start(xt[:, k, t0:t0 + tn], xTr[:, k, t0:t0 + tn])

            ht = hp.tile([_P, MF, n_pad], mm_dt, tag="ht")

            def l1_group(m, sub):
                """One layer-1 m-group over the token tiles in `sub`
                (k-outer: Ldweights dedupe collapses each k's run)."""
                pss = {
                    i: pp.tile([_P, tn], f32, tag="ps", name=f"ps_{m}_{i}")
                    for i, (t0, tn) in sub
                }
                for k in range(KH):
                    for i, (t0, tn) in sub:
                        nc.tensor.matmul(
                            pss[i][:],
                            w1s[:, k, m * _P:(m + 1) * _P],
                            xt[:, k, t0:t0 + tn],
                            start=(k == 0),
                            stop=(k == KH - 1),
                        )
                for i, (t0, tn) in sub:
                    nc.scalar.activation(
                        ht[:, m, t0:t0 + tn], pss[i][:], AF.Relu, bias=b1s[:, m:m + 1]
                    )

            all_tiles = list(enumerate(tiles))
            if cold:
                # Hybrid head: tile 0 alone first (its x lands ~1us in; the
                # 13.6us of PE work on it hides the rest of the x/W DMA),
                # then the remaining tiles weight-stationary.
                for m in range(MF):
                    l1_group(m, all_tiles[:1])
                for m in range(MF):
                    l1_group(m, all_tiles[1:])
            else:
                for m in range(MF):
                    l1_group(m, all_tiles)

            yt = yp.tile([_P, KH, n_pad], mm_dt, tag="yt")

            def l2_group(m2, sub, k_inner=False):
                pss2 = {
                    i: pp.tile([_P, tn], f32, tag="ps", name=f"ps2_{m2}_{i}")
                    for i, (t0, tn) in sub
                }
                order = (
                    [(i, t0, tn, k2) for i, (t0, tn) in sub for k2 in range(MF)]
                    if k_inner
                    else [(i, t0, tn, k2) for k2 in range(MF) for i, (t0, tn) in sub]
                )
                done = set()
                for i, t0, tn, k2 in order:
                    nc.tensor.matmul(
                        pss2[i][:],
                        w2s[:, k2, m2 * _P:(m2 + 1) * _P],
                        ht[:, k2, t0:t0 + tn],
                        start=(k2 == 0),
                        stop=(k2 == MF - 1),
                    )
                    if k_inner and k2 == MF - 1 and i not in done:
                        done.add(i)
                        nc.scalar.activation(
                            yt[:, m2, t0:t0 + tn], pss2[i][:], AF.Identity, bias=b2s[:, m2:m2 + 1]
                        )
                        nc.scalar.dma_start(yTr[:, m2, t0:t0 + tn], yt[:, m2, t0:t0 + tn])
                if not k_inner:
                    for i, (t0, tn) in sub:
                        nc.scalar.activation(
                            yt[:, m2, t0:t0 + tn], pss2[i][:], AF.Identity, bias=b2s[:, m2:m2 + 1]
                        )
                        nc.scalar.dma_start(yTr[:, m2, t0:t0 + tn], yt[:, m2, t0:t0 + tn])

            for m2 in range(KH):
                # Cold pass: the last m2-group runs tile-major (k-inner) so
                # only the final small tile's evac+DMA trails the last MM.
                l2_group(m2, all_tiles, k_inner=(cold and m2 == KH - 1))

    nc.compile()
    if mm_dtype_name not in ("float32", "float32r"):
        n_removed = _dedupe_ldweights(nc)
        import os as _os
        if _os.environ.get("DEBUG_LDW"):
            print(f"[kernel v3] deduped {n_removed} Ldweights")
    return nc


def _build(n_pad, mm_dtype_name, repeats=1, loop=1, style="stream"):
    if style == "v2":
        return _build_v2(n_pad, mm_dtype_name, repeats, loop)
    if style == "v3":
        return _build_v3(n_pad, mm_dtype_name, repeats, loop)
    import os as _os
    _dev = _os.environ.get("BSPFF_DEV") == "1"
    sub_n = int(_os.environ.get("SUB_N", "512")) if _dev else 512
    skip_act = _dev and _os.environ.get("SKIP_ACT") == "1"
    skip_ydma = _dev and _os.environ.get("SKIP_YDMA") == "1"
    skip_xdma = _dev and _os.environ.get("SKIP_XDMA") == "1"
    evac = _os.environ.get("EVAC", "act") if _dev else "act"
    w_in_loop = _dev and _os.environ.get("W_IN_LOOP") == "1"

    from contextlib import ExitStack, nullcontext

    import concourse.bass as bass
    import concourse.mybir as mybir
    import concourse.tile as tile
    from concourse import bacc

    f32 = mybir.dt.float32
    mm_dt = getattr(mybir.dt, mm_dtype_name)
    AF = mybir.ActivationFunctionType
    KH = _H // _P   # 4  K-subtiles for layer 1 / M-tiles for layer 2
    MF = _F // _P   # 16 M-tiles for layer 1 / K-subtiles for layer 2

    nc = bacc.Bacc("TRN2", target_bir_lowering=False, debug=False, num_devices=_NCORES)
    xT = nc.dram_tensor("xT", [_H, n_pad], mm_dt, kind="ExternalInput").ap()
    w1 = nc.dram_tensor("w1", [_H, _F], mm_dt, kind="ExternalInput").ap()
    w2 = nc.dram_tensor("w2", [_F, _H], mm_dt, kind="ExternalInput").ap()
    b1 = nc.dram_tensor("b1", [_P, MF], f32, kind="ExternalInput").ap()
    b2 = nc.dram_tensor("b2", [_P, KH], f32, kind="ExternalInput").ap()
    yT = nc.dram_tensor("yT", [_H, n_pad], f32, kind="ExternalOutput").ap()

    with tile.TileContext(nc) as tc, ExitStack() as ctx:
        grouped = style == "grouped"
        consts = ctx.enter_context(tc.tile_pool(name="consts", bufs=1))
        xp = ctx.enter_context(tc.tile_pool(name="xp", bufs=2 if grouped else 3))
        hp = ctx.enter_context(tc.tile_pool(name="hp", bufs=1 if grouped else 2))
        yp = ctx.enter_context(tc.tile_pool(name="yp", bufs=2 if grouped else 3))
        import os as _os2
        _onetag = _os2.environ.get("PSUM_ONETAG") == "1" and _os.environ.get("BSPFF_DEV") == "1"
        pp = ctx.enter_context(tc.tile_pool(name="pp", bufs=8 if (grouped or _onetag) else 4, space="PSUM"))

        w1s = consts.tile([_P, KH, _F], mm_dt)
        w2s = consts.tile([_P, MF, _H], mm_dt)
        w1r = w1.rearrange("(ko p) f -> p ko f", p=_P)
        w2r = w2.rearrange("(ko p) h -> p ko h", p=_P)

        def load_weights():
            # Chunk weight loads by output-column slice: the m-th matmul group
            # only needs its own 128-wide slice, so compute starts ~1-2 us in.
            # W2 chunks are interleaved between W1 chunks so layer-2's first
            # weights aren't queued behind all 4 MB of W1 on the DMA queues
            # (layer 2 of token-tile 0 starts ~17 us into a cold pass).
            for m in range(MF):
                nc.sync.dma_start(w1s[:, :, m * _P:(m + 1) * _P], w1r[:, :, m * _P:(m + 1) * _P])
                if m % 4 == 3:
                    m2 = m // 4
                    nc.sync.dma_start(w2s[:, :, m2 * _P:(m2 + 1) * _P], w2r[:, :, m2 * _P:(m2 + 1) * _P])

        if not w_in_loop:
            load_weights()
        b1s = consts.tile([_P, MF], f32)
        nc.sync.dma_start(b1s[:], b1)
        b2s = consts.tile([_P, KH], f32)
        nc.sync.dma_start(b2s[:], b2)

        xTr = xT.rearrange("(ko p) n -> p ko n", p=_P)
        yTr = yT.rearrange("(mo p) n -> p mo n", p=_P)

        tiles = _token_tiles(n_pad)
        loop_cm = (
            tc.For_i(0, loop, 1, hint_engines=(mybir.EngineType.PE, mybir.EngineType.Activation, mybir.EngineType.SP))
            if loop > 1
            else nullcontext()
        )
        with loop_cm:
          for _rep in range(repeats):
            if w_in_loop:
                load_weights()
            if style == "grouped":
                # Tokens-inner order: one weight tile feeds all token tiles
                # back-to-back, so redundant Ldweights can be dropped.
                xt = xp.tile([_P, KH, n_pad], mm_dt, tag="xt")
                for k in range(KH):
                    nc.sync.dma_start(xt[:, k, :], xTr[:, k, :])
                ht = hp.tile([_P, MF, n_pad], mm_dt, tag="ht")
                for m in range(MF):
                    pss = [
                        pp.tile([_P, tn], f32, tag="ps", name=f"ps_{m}_{i}")
                        for i, (t0, tn) in enumerate(tiles)
                    ]
                    for k in range(KH):
                        for i, (t0, tn) in enumerate(tiles):
                            nc.tensor.matmul(
                                pss[i][:],
                                w1s[:, k, m * _P:(m + 1) * _P],
                                xt[:, k, t0:t0 + tn],
                                start=(k == 0),
                                stop=(k == KH - 1),
                            )
                    for i, (t0, tn) in enumerate(tiles):
                        nc.scalar.activation(
                            ht[:, m, t0:t0 + tn], pss[i][:], AF.Relu, bias=b1s[:, m:m + 1]
                        )
                yt = yp.tile([_P, KH, n_pad], f32, tag="yt")
                for m2 in range(KH):
                    pss2 = [
                        pp.tile([_P, tn], f32, tag="ps", name=f"ps2_{m2}_{i}")
                        for i, (t0, tn) in enumerate(tiles)
                    ]
                    for k2 in range(MF):
                        for i, (t0, tn) in enumerate(tiles):
                            nc.tensor.matmul(
                                pss2[i][:],
                                w2s[:, k2, m2 * _P:(m2 + 1) * _P],
                                ht[:, k2, t0:t0 + tn],
                                start=(k2 == 0),
                                stop=(k2 == MF - 1),
                            )
                    for i, (t0, tn) in enumerate(tiles):
                        nc.scalar.activation(
                            yt[:, m2, t0:t0 + tn], pss2[i][:], AF.Identity, bias=b2s[:, m2:m2 + 1]
                        )
                for m2 in range(KH):
                    nc.sync.dma_start(yTr[:, m2, :], yt[:, m2, :])
                continue

            for t0, tn in tiles:
                sl = slice(t0, t0 + tn)
                nq = (tn + sub_n - 1) // sub_n
                xt = xp.tile([_P, KH, tn], mm_dt, tag="xt")
                if not skip_xdma:
                    # Chunked by k-subtile: 4 queues in parallel, and matmul
                    # group (m=0, k=0) only waits on the first quarter.
                    for k in range(KH):
                        nc.sync.dma_start(xt[:, k, :], xTr[:, k, sl])

                ht = hp.tile([_P, MF, tn], mm_dt, tag="ht")
                for m in range(MF):
                    ps = pp.tile([_P, tn], f32, tag="ps" if _onetag else "ps1", name="ps1")
                    for k in range(KH):
                        for q in range(nq):
                            qs = slice(q * sub_n, min((q + 1) * sub_n, tn))
                            nc.tensor.matmul(
                                ps[:, qs],
                                w1s[:, k, m * _P:(m + 1) * _P],
                                xt[:, k, qs],
                                start=(k == 0 and q == 0),
                                stop=(k == KH - 1 and q == nq - 1),
                                skip_group_check=True,
                            )
                    if not skip_act:
                        use_dve = evac == "dve" or (evac == "split" and m % 2 == 1)
                        if use_dve:
                            nc.vector.tensor_scalar(
                                ht[:, m, :], ps[:], b1s[:, m:m + 1], 0.0,
                                mybir.AluOpType.add, mybir.AluOpType.max,
                            )
                        else:
                            nc.scalar.activation(ht[:, m, :], ps[:], AF.Relu, bias=b1s[:, m:m + 1])

                yt = yp.tile([_P, KH, tn], f32, tag="yt")
                for m2 in range(KH):
                    ps2 = pp.tile([_P, tn], f32, tag="ps" if _onetag else "ps2", name="ps2")
                    for k2 in range(MF):
                        for q in range(nq):
                            qs = slice(q * sub_n, min((q + 1) * sub_n, tn))
                            nc.tensor.matmul(
                                ps2[:, qs],
                                w2s[:, k2, m2 * _P:(m2 + 1) * _P],
                                ht[:, k2, qs],
                                start=(k2 == 0 and q == 0),
                                stop=(k2 == MF - 1 and q == nq - 1),
                                skip_group_check=True,
                            )
                    if not skip_act:
                        use_dve = evac == "dve" or (evac == "split" and m2 % 2 == 1)
                        if use_dve:
                            nc.vector.tensor_scalar_add(yt[:, m2, :], ps2[:], b2s[:, m2:m2 + 1])
                        else:
                            nc.scalar.activation(yt[:, m2, :], ps2[:], AF.Identity, bias=b2s[:, m2:m2 + 1])
                if not skip_ydma:
                    nc.sync.dma_start(yTr[:, :, sl], yt[:])

    nc.compile()
    if style == "grouped":
        n_removed = _dedupe_ldweights(nc)
        import os as _os
        if _os.environ.get("DEBUG_LDW"):
            print(f"[kernel] deduped {n_removed} Ldweights")
    return nc


_MM_DTYPE = "float16"
_MM_STYLE = "v3"


def _get_nc(n_pad, mm_dtype_name, repeats=1, loop=1, style="stream"):
    key = (n_pad, mm_dtype_name, repeats, loop, style)
    if key not in _NC_CACHE:
        _NC_CACHE[key] = _build(n_pad, mm_dtype_name, repeats, loop, style)
    return _NC_CACHE[key]


def _np_mm_dtype(mm_dtype_name):
    if mm_dtype_name == "bfloat16":
        import ml_dtypes

        return ml_dtypes.bfloat16
    if mm_dtype_name == "float16":
        return np.float16
    return np.float32


def _prepare(x, b_seq, W1, B1, W2, B2, mm_dtype_name):
    """Host-side routing: returns (idx_per_core, n_pad, in_maps)."""
    np_dt = _np_mm_dtype(mm_dtype_name)
    x = np.asarray(x)
    flat_x = np.ascontiguousarray(x.reshape(-1, _H), dtype=np.float32)
    bs = np.asarray(b_seq).reshape(-1)

    # Route: behavior b -> cores 2b and 2b+1, tokens split evenly.
    idx_per_core = []
    for b in range(_NB):
        idx = np.nonzero(bs == b + 1)[0]
        h = (len(idx) + 1) // 2
        idx_per_core.append(idx[:h])
        idx_per_core.append(idx[h:])
    nmax = max(len(i) for i in idx_per_core)
    n_pad = ((max(256, nmax) + 15) // 16) * 16

    in_maps = []
    for c in range(_NCORES):
        beh = c // 2
        idx = idx_per_core[c]
        xT = np.zeros((_H, n_pad), np_dt)
        if len(idx):
            xT[:, :len(idx)] = flat_x[idx].T.astype(np_dt)
        in_maps.append({
            "xT": xT,
            "w1": np.ascontiguousarray(np.asarray(W1[beh]).astype(np_dt)),
            "w2": np.ascontiguousarray(np.asarray(W2[beh]).astype(np_dt)),
            "b1": np.ascontiguousarray(np.asarray(B1[beh], dtype=np.float32).reshape(_F // _P, _P).T),
            "b2": np.ascontiguousarray(np.asarray(B2[beh], dtype=np.float32).reshape(_H // _P, _P).T),
        })
    return idx_per_core, n_pad, in_maps


def kernel(x, b_seq, W1, B1, W2, B2, _repeats=1):
    global LAST_RESULTS
    import os

    from concourse.bass_utils import run_bass_kernel_spmd

    _dev = os.environ.get("BSPFF_DEV") == "1"
    mm_dtype = os.environ.get("MM_DTYPE", _MM_DTYPE) if _dev else _MM_DTYPE
    style = os.environ.get("MM_STYLE", _MM_STYLE) if _dev else _MM_STYLE
    idx_per_core, n_pad, in_maps = _prepare(x, b_seq, W1, B1, W2, B2, mm_dtype)
    nc = _get_nc(n_pad, mm_dtype, _repeats, style=style)

    res = run_bass_kernel_spmd(nc, in_maps, core_ids=list(range(_NCORES)))
    LAST_RESULTS = res

    out = np.zeros((_B * _T, _H), np.float32)
    for c in range(_NCORES):
        idx = idx_per_core[c]
        if len(idx):
            out[idx] = res.results[c]["yT"][:, :len(idx)].T
    return out.reshape(_B, _T, _H)

